# revision 8
# baseline (speedup 1.0000x reference)
"""Trainium2 Bass kernel for nn_ModelPaperBaseline_bin (binarized CNN).

Contract: kernel(**inputs) takes FULL unsharded inputs (batch 65536) and
returns the FULL (65536, 1) float32 output. Internally shards the batch
across 8 NeuronCores (pure data parallel), runs one SPMD Bass program.

Network (per sample):
  x (4,16) -> conv0 1x1 (bin W) -> BN -> sign -> y0 (32,16)   [shortcut]
  4x { conv3x1 pad1 (bin W) -> BN -> sign -> + shortcut }
  flatten (512) -> fc1 (bin W) -> BN -> sign -> (64)
  fc2 (bin W) -> BN -> sign -> (64) -> fc3 (bin W) -> sigmoid -> (1)

All BN+bias+bintanh stages fold into per-channel thresholds (BN scale>0).
After conv0's sign, every activation is an exact small integer, so bf16
matmuls are bit-exact; accumulation is fp32 in PSUM.

On-chip layout: partition p = 32*j + c  (j = position-in-quad, c = channel),
column n = g*S + b (g = quad index 0..3, b = sample-in-chunk). W=16 = 4 quads.
Conv = banded 128x128 matmul (within-quad taps) + two 32x32 tile-positioned
matmuls with column-shifted rhs (cross-quad taps); zero padding falls out of
restricting the shifted matmuls' column ranges.
"""

import os
import sys

sys.path.insert(0, "/opt/trn_rl_repo")

import numpy as np
import ml_dtypes

BF16 = ml_dtypes.bfloat16
EPS = 0.01

B_TOTAL = 65536
N_CORES = 8
B_CORE = B_TOTAL // N_CORES  # 8192
S = 256          # samples per chunk
CF = 4 * S       # columns per activation tile (4 quads * S)
W = 16
C = 32
H = 64

# which of the 4 residual conv layers use ScalarE Sign (others: VectorE is_ge)
ACT_LAYERS = (0, 2)


def _sgn(w):
    return np.where(w >= 0, 1.0, -1.0).astype(np.float32)


def prepare_host_tensors(inp):
    """Fold BN into thresholds, binarize + rearrange weights into lhsT layouts."""
    f32 = np.float32
    out = {}

    # conv0: z0[32j+c', gS+b] = sum_cin sgn(w0)[c',cin] * x[b,cin,4g+j]
    w0s = _sgn(inp["conv0_w"][:, :, 0])             # [cout=32, cin=4]
    s0 = inp["bn0_g"] / np.sqrt(inp["bn0_v"] + EPS)
    thr0 = inp["bn0_m"] - inp["bn0_b"] / s0 - inp["conv0_b"]   # [32]
    w0g = np.zeros((4, 64, 128), f32)               # [g, (16*cin+i), (32j'+c')]
    for g in range(4):
        for jp in range(4):
            i = 4 * g + jp
            for cin in range(4):
                w0g[g, 16 * cin + i, 32 * jp:32 * jp + 32] = w0s[:, cin]
    out["w0g"] = np.ascontiguousarray(w0g.transpose(1, 0, 2))  # [64, 4, 128]
    out["nth0"] = np.tile(-thr0, 4).reshape(128, 1).astype(f32)

    # residual conv blocks
    ws = _sgn(inp["convs_w"])                       # [4, cout, cin, 3]
    sl = inp["bns_g"] / np.sqrt(inp["bns_v"] + EPS)
    thr = inp["bns_m"] - inp["bns_b"] / sl - inp["convs_b"]    # [4, 32]
    wm1 = np.zeros((4, 128, 128), f32)
    for l in range(4):
        for j in range(4):
            for jp in range(4):
                if abs(jp - j) <= 1:
                    # lhsT[32j+c, 32jp+c'] = W[c', c, j-jp+1]
                    # (input pos i = 4g+j, output pos i' = 4g+jp, tap k = i-i'+1)
                    wm1[l, 32 * j:32 * j + 32, 32 * jp:32 * jp + 32] = \
                        ws[l, :, :, j - jp + 1].T
    # boundary taps: rows 96:128 = left tap (dx=-1 -> W[...,0]), rows 0:32 = right tap (W[...,2])
    wmb = np.zeros((4, 128, 32), f32)
    for l in range(4):
        wmb[l, 96:128, :] = ws[l, :, :, 0].T
        wmb[l, 0:32, :] = ws[l, :, :, 2].T
    out["wm1"] = np.ascontiguousarray(wm1.transpose(1, 0, 2)).astype(BF16)  # [128,4,128]
    out["wmb"] = np.ascontiguousarray(wmb.transpose(1, 0, 2)).astype(BF16)  # [128,4,32]
    thl = np.stack([np.tile(thr[l], 4) for l in range(4)], 1)  # [128, 4]
    out["thl"] = thl.astype(f32)       # raw threshold (VectorE is_ge)
    out["nthl"] = (-thl).astype(f32)   # negated (ScalarE Sign bias)

    # fc1: h[h', b] = sum_{j,c,g} sgn(fc1_w)[h', c*16+4g+j] * y4[32j+c, gS+b]
    f1s = _sgn(inp["fc1_w"])                        # [64, 512]
    s5 = inp["bn5_g"] / np.sqrt(inp["bn5_v"] + EPS)
    thr5 = inp["bn5_m"] - inp["bn5_b"] / s5 - inp["fc1_b"]
    wf1 = np.zeros((4, 128, 64), f32)
    for g in range(4):
        for j in range(4):
            for c in range(C):
                wf1[g, 32 * j + c, :] = f1s[:, c * 16 + 4 * g + j]
    out["wf1"] = np.ascontiguousarray(wf1.transpose(1, 0, 2)).astype(BF16)  # [128,4,64]
    out["nth5"] = (-thr5).reshape(64, 1).astype(f32)

    f2s = _sgn(inp["fc2_w"])
    s6 = inp["bn6_g"] / np.sqrt(inp["bn6_v"] + EPS)
    thr6 = inp["bn6_m"] - inp["bn6_b"] / s6 - inp["fc2_b"]
    out["wf2"] = f2s.T.copy().astype(BF16)          # [h1, h2]
    out["nth6"] = (-thr6).reshape(64, 1).astype(f32)

    out["wf3"] = _sgn(inp["fc3_w"]).T.copy().astype(BF16)  # [64, 1]
    out["b7"] = inp["fc3_b"].reshape(1, 1).astype(f32)
    return out


def build_nc(b_core=B_CORE, s=S):
    from concourse import bass, mybir, tile

    cf = 4 * s
    nchunk = b_core // s
    f32 = mybir.dt.float32
    bf16 = mybir.dt.bfloat16
    AF = mybir.ActivationFunctionType
    ALU = mybir.AluOpType

    nc = bass.Bass()
    x_d = nc.dram_tensor("x", [b_core, 4, 16], f32, kind="ExternalInput")
    w0g_d = nc.dram_tensor("w0g", [64, 4, 128], f32, kind="ExternalInput")
    nth0_d = nc.dram_tensor("nth0", [128, 1], f32, kind="ExternalInput")
    wm1_d = nc.dram_tensor("wm1", [128, 4, 128], bf16, kind="ExternalInput")
    wmb_d = nc.dram_tensor("wmb", [128, 4, 32], bf16, kind="ExternalInput")
    thl_d = nc.dram_tensor("thl", [128, 4], f32, kind="ExternalInput")
    nthl_d = nc.dram_tensor("nthl", [128, 4], f32, kind="ExternalInput")
    wf1_d = nc.dram_tensor("wf1", [128, 4, 64], bf16, kind="ExternalInput")
    nth5_d = nc.dram_tensor("nth5", [64, 1], f32, kind="ExternalInput")
    wf2_d = nc.dram_tensor("wf2", [64, 64], bf16, kind="ExternalInput")
    nth6_d = nc.dram_tensor("nth6", [64, 1], f32, kind="ExternalInput")
    wf3_d = nc.dram_tensor("wf3", [64, 1], bf16, kind="ExternalInput")
    b7_d = nc.dram_tensor("b7", [1, 1], f32, kind="ExternalInput")
    out_d = nc.dram_tensor("out", [1, b_core], f32, kind="ExternalOutput")

    with tile.TileContext(nc) as tc:
        with (
            tc.tile_pool(name="const", bufs=1) as constp,
            tc.tile_pool(name="xin", bufs=3) as xpool,
            tc.tile_pool(name="acts", bufs=3) as apool,
            tc.tile_pool(name="sgn", bufs=2) as spool,
            tc.tile_pool(name="fcact", bufs=3) as fpool,
            tc.tile_pool(name="outs", bufs=3) as opool,
            tc.tile_pool(name="cpsum", bufs=3, space="PSUM") as cpsum,
            tc.tile_pool(name="fpsum", bufs=2, space="PSUM") as fpsum,
        ):
            # ---- constants to SBUF (once) ----
            w0t = constp.tile([64, 4 * 128], f32, tag="w0t")
            nc.sync.dma_start(w0t[:], w0g_d.rearrange("p g f -> p (g f)"))
            nth0t = constp.tile([128, 1], f32, tag="nth0t")
            nc.sync.dma_start(nth0t[:], nth0_d[:])
            wm1t = constp.tile([128, 4 * 128], bf16, tag="wm1t")
            nc.sync.dma_start(wm1t[:], wm1_d.rearrange("p l f -> p (l f)"))
            wmbt = constp.tile([128, 4 * 32], bf16, tag="wmbt")
            nc.sync.dma_start(wmbt[:], wmb_d.rearrange("p l f -> p (l f)"))
            thlt = constp.tile([128, 4], f32, tag="thlt")
            nc.sync.dma_start(thlt[:], thl_d[:])
            nthlt = constp.tile([128, 4], f32, tag="nthlt")
            nc.sync.dma_start(nthlt[:], nthl_d[:])
            wf1t = constp.tile([128, 4 * 64], bf16, tag="wf1t")
            nc.sync.dma_start(wf1t[:], wf1_d.rearrange("p g f -> p (g f)"))
            nth5t = constp.tile([64, 1], f32, tag="nth5t")
            nc.sync.dma_start(nth5t[:], nth5_d[:])
            wf2t = constp.tile([64, 64], bf16, tag="wf2t")
            nc.sync.dma_start(wf2t[:], wf2_d[:])
            nth6t = constp.tile([64, 1], f32, tag="nth6t")
            nc.sync.dma_start(nth6t[:], nth6_d[:])
            wf3t = constp.tile([64, 1], bf16, tag="wf3t")
            nc.sync.dma_start(wf3t[:], wf3_d[:])
            b7t = constp.tile([1, 1], f32, tag="b7t")
            nc.sync.dma_start(b7t[:], b7_d[:])

            nbank = cf // 512 if cf >= 512 else 1  # psum-bank col granularity
            bank_cols = min(cf, 512)

            for k in range(nchunk):
                # ---- load x chunk: [64=(16*cin+i), s] ----
                xt = xpool.tile([64, s], f32, tag="xt")
                nc.sync.dma_start(
                    xt[:], x_d[k * s:(k + 1) * s].rearrange("b c i -> (c i) b"))

                # ---- conv0 ----
                z0 = cpsum.tile([128, cf], f32, tag="zconv")
                for g in range(4):
                    lo = g * s
                    first = (lo % 512) == 0
                    last = ((lo + s) % 512) == 0 or g == 3
                    nc.tensor.matmul(z0[:, lo:lo + s],
                                     w0t[:, g * 128:(g + 1) * 128], xt[:],
                                     start=first, stop=last)
                s0t = spool.tile([128, cf], bf16, tag="s0")
                nc.scalar.activation(s0t[:], z0[:], AF.Sign, bias=nth0t[:, 0:1])
                s0m1 = spool.tile([128, cf], bf16, tag="s0m1")
                nc.vector.tensor_scalar(s0m1[:], s0t[:], -1.0, None, ALU.add)

                y = s0t
                # ---- 4 residual conv blocks ----
                for l in range(4):
                    zp = cpsum.tile([128, cf], f32, tag="zconv")
                    wl = wm1t[:, l * 128:(l + 1) * 128]
                    w2 = wmbt[96:128, l * 32:(l + 1) * 32]  # left tap
                    w3 = wmbt[0:32, l * 32:(l + 1) * 32]    # right tap
                    for bk in range(nbank):
                        lo = bk * bank_cols
                        hi = lo + bank_cols
                        # main banded matmul (within-quad taps): clears + fills bank
                        nc.tensor.matmul(zp[:, lo:hi], wl, y[:, lo:hi],
                                         start=True, stop=True)
                        # boundary taps accumulate per-element on top (stop is
                        # sim-only bookkeeping; has_written bits are set by m1)
                        # left tap: out cols [s, cf) <- y[96:128, col-s]
                        l2lo, l2hi = max(lo, s), hi
                        if l2hi > l2lo:
                            nc.tensor.matmul(zp[0:32, l2lo:l2hi], w2,
                                             y[96:128, l2lo - s:l2hi - s],
                                             start=False, stop=False,
                                             tile_position=(96, 0),
                                             skip_group_check=True)
                        # right tap: out cols [0, 3s) <- y[0:32, col+s]
                        l3lo, l3hi = lo, min(hi, 3 * s)
                        if l3hi > l3lo:
                            nc.tensor.matmul(zp[96:128, l3lo:l3hi], w3,
                                             y[0:32, l3lo + s:l3hi + s],
                                             start=False, stop=False,
                                             tile_position=(0, 96),
                                             skip_group_check=True)
                    st = spool.tile([128, cf], bf16, tag="st")
                    ynew = apool.tile([128, cf], bf16, tag="y")
                    if l in ACT_LAYERS:
                        nc.scalar.activation(st[:], zp[:], AF.Sign,
                                             bias=nthlt[:, l:l + 1])
                        nc.vector.tensor_add(ynew[:], st[:], s0t[:])
                    else:
                        nc.vector.tensor_scalar(st[:], zp[:], thlt[:, l:l + 1],
                                                2.0, ALU.is_ge, ALU.mult)
                        nc.vector.tensor_add(ynew[:], st[:], s0m1[:])
                    y = ynew

                # ---- fc1 ----
                h1p = fpsum.tile([64, s], f32, tag="hp")
                for g in range(4):
                    nc.tensor.matmul(h1p[:], wf1t[:, g * 64:(g + 1) * 64],
                                     y[:, g * s:(g + 1) * s],
                                     start=(g == 0), stop=(g == 3))
                h1 = fpool.tile([64, s], bf16, tag="h1")
                nc.scalar.activation(h1[:], h1p[:], AF.Sign, bias=nth5t[:, 0:1])
                # ---- fc2 ----
                h2p = fpsum.tile([64, s], f32, tag="hp")
                nc.tensor.matmul(h2p[:], wf2t[:], h1[:], start=True, stop=True)
                h2 = fpool.tile([64, s], bf16, tag="h2")
                nc.scalar.activation(h2[:], h2p[:], AF.Sign, bias=nth6t[:, 0:1])
                # ---- fc3 + sigmoid ----
                op_ = fpsum.tile([1, s], f32, tag="hp")
                nc.tensor.matmul(op_[:], wf3t[:], h2[:], start=True, stop=True)
                ot = opool.tile([1, s], f32, tag="ot")
                nc.scalar.activation(ot[:], op_[:], AF.Sigmoid, bias=b7t[0:1, 0:1])
                nc.sync.dma_start(out_d[0:1, k * s:(k + 1) * s], ot[:])

    nc.finalize()
    return nc


_NC_CACHE = {}
LAST_EXEC_NS = None
_PATCHED = False


def _split_multiwait_json(bir_bytes):
    """Walrus in this toolchain only supports ONE sync-wait per instruction.
    Split any instruction carrying N>1 waits into N-1 preceding single-wait
    NoOps on the same engine (waits are monotone sem-ge checks, so order is
    irrelevant and the split is semantics-preserving)."""
    import json as _json
    d = _json.loads(bir_bytes)
    nsplit = 0
    for fn in d.get("functions", []):
        for blk in fn.get("blocks", []):
            out = []
            for inst in blk.get("instructions", []):
                si = inst.get("sync_info")
                waits = (si or {}).get("on_wait") or []
                if len(waits) > 1:
                    for wi, w in enumerate(waits[:-1]):
                        out.append({
                            "name": f"{inst['name']}-ws{wi}",
                            "opcode": "NoOp",
                            "engine": inst["engine"],
                            "ins": [],
                            "outs": [],
                            "debug": inst.get("debug", 0),
                            "sync_info": {"on_update": [], "on_wait": [w]},
                        })
                        nsplit += 1
                    si["on_wait"] = [waits[-1]]
                out.append(inst)
            blk["instructions"] = out
    if nsplit:
        print(f"[kernel] split {nsplit} extra sync-waits into NoOps",
              file=sys.stderr)
    return _json.dumps(d).encode()


def _install_patches():
    global _PATCHED
    if _PATCHED:
        return
    from concourse import bass_utils, bass2jax
    orig = bass_utils.compile_bir_kernel

    def patched(bir_json, tmpdir, neff_name="file.neff", **kw):
        if isinstance(bir_json, str):
            bir_json = bir_json.encode()
        return orig(_split_multiwait_json(bir_json), tmpdir, neff_name, **kw)

    bass_utils.compile_bir_kernel = patched
    bass2jax.compile_bir_kernel = patched
    _PATCHED = True


def kernel(**inputs):
    _install_patches()
    from concourse.bass_utils import run_bass_kernel_spmd

    x = np.asarray(inputs["x"], np.float32)
    b_total = x.shape[0]
    b_core = b_total // N_CORES
    host = prepare_host_tensors({k: np.asarray(v) for k, v in inputs.items()})

    key = (b_core, S)
    if key not in _NC_CACHE:
        _NC_CACHE[key] = build_nc(b_core, S)
    nc = _NC_CACHE[key]

    in_maps = []
    for ci in range(N_CORES):
        m = {"x": np.ascontiguousarray(x[ci * b_core:(ci + 1) * b_core])}
        m.update(host)
        in_maps.append(m)

    trace = os.environ.get("KTRACE", "0") == "1"
    res = run_bass_kernel_spmd(nc, in_maps, core_ids=list(range(N_CORES)),
                               trace=trace)
    global LAST_EXEC_NS
    LAST_EXEC_NS = res.exec_time_ns
    outs = [res.results[i]["out"].reshape(-1) for i in range(N_CORES)]
    return np.concatenate(outs).reshape(b_total, 1).astype(np.float32)


if __name__ == "__main__":
    rng = np.random.default_rng(0)
    demo = {
        "x": rng.standard_normal((B_TOTAL, 4, 16), dtype=np.float32),
        "conv0_w": rng.standard_normal((32, 4, 1), dtype=np.float32),
        "conv0_b": rng.standard_normal(32, dtype=np.float32),
        "bn0_g": rng.uniform(0.5, 1.5, 32).astype(np.float32),
        "bn0_b": rng.standard_normal(32, dtype=np.float32),
        "bn0_m": rng.standard_normal(32, dtype=np.float32),
        "bn0_v": np.ones(32, np.float32),
        "convs_w": rng.standard_normal((4, 32, 32, 3), dtype=np.float32),
        "convs_b": rng.standard_normal((4, 32), dtype=np.float32),
        "bns_g": rng.uniform(0.5, 1.5, (4, 32)).astype(np.float32),
        "bns_b": rng.standard_normal((4, 32), dtype=np.float32),
        "bns_m": rng.standard_normal((4, 32), dtype=np.float32),
        "bns_v": np.ones((4, 32), np.float32),
        "fc1_w": rng.standard_normal((64, 512), dtype=np.float32),
        "fc1_b": rng.standard_normal(64, dtype=np.float32),
        "bn5_g": rng.uniform(0.5, 1.5, 64).astype(np.float32),
        "bn5_b": rng.standard_normal(64, dtype=np.float32),
        "bn5_m": rng.standard_normal(64, dtype=np.float32),
        "bn5_v": np.ones(64, np.float32),
        "fc2_w": rng.standard_normal((64, 64), dtype=np.float32),
        "fc2_b": rng.standard_normal(64, dtype=np.float32),
        "bn6_g": rng.uniform(0.5, 1.5, 64).astype(np.float32),
        "bn6_b": rng.standard_normal(64, dtype=np.float32),
        "bn6_m": rng.standard_normal(64, dtype=np.float32),
        "bn6_v": np.ones(64, np.float32),
        "fc3_w": rng.standard_normal((1, 64), dtype=np.float32),
        "fc3_b": rng.standard_normal(1, dtype=np.float32),
    }
    o = kernel(**demo)
    print(o.shape, o[:4, 0])
